# revision 8
# baseline (speedup 1.0000x reference)
"""AxileAttention Trainium2 kernel, v3 (self-contained).

Problem: x[8,64,256,256] fp32; per-channel weights *_w[64,256,256], biases *_b[64,256,256]:
    q = einsum("bchw,cwv->bchv", x, query_w) + query_b   (same for k, v)
    out = softmax(q*k, axis=-1) * v

Strategy (8 NeuronCores, SPMD): shard channel axis C=64 -> 8 channels/core.
Per (batch, channel) pair:
  * 3 PSUM banks Q=[q_m0|q_m1], K=[k_m0|k_m1], V=[v_m0|v_m1] (h = 2j+m
    interleave; partition j holds rows 2j, 2j+1). Biases preloaded via
    f32r identity matmuls; 12 f32r data matmuls accumulate on top.
  * ScalarE evacuates K -> SBUF; one fused 512-wide DVE op computes
    s_neg = -(q*k) and the union -rowmax; 2x ScalarE Exp (shared union max
    is a valid softmax shift) accumulate per-half row sums; DVE reciprocal;
    2x DVE scalar_tensor_tensor out = (p * 1/sum) * v reading v straight
    from PSUM, writing bf16.
  * DMAs are all HWDGE (nc.sync), batched per channel: x (all 8 batches),
    packed weights [128,2,3,256], packed biases [128,2,3,256], bf16 output
    (all 8 batches). f32r is bit-identical to fp32 so inputs are declared
    f32r in DRAM and copied raw (no SWDGE cast DMAs).
"""
import sys

sys.path.insert(0, "/opt/trn_rl_repo")

import numpy as np

import concourse.bacc as bacc
import concourse.tile as tile
import concourse.dve_ops as dve_ops
from concourse import mybir
from concourse.masks import make_identity
from concourse.dve_spec import C0, C1, Spec, Src0, Src1, lower, minn, _has_src1
from concourse.dve_uop import DveOpSpec

F32 = mybir.dt.float32
F32R = mybir.dt.float32r
BF16 = mybir.dt.bfloat16

B = 8        # batch
C = 64       # channels total
CCH = 8      # channels per core
NCORES = 8
HP = 2       # h partition-tiles (h = 2j + m interleave)
KT = 2       # w partition-tiles (w = 2p + k interleave)
H = W = V = 256


def _make_ttr_min():
    """Custom DVE op: out = (in0*in1)*s1 ; accum_out = min(s0, row-min of out).
    Called with s1=-1, s0=+BIG: out = -(q*k), accum = -rowmax(q*k)."""
    name = "TTR_MIN_NEG_ANT"
    for op in dve_ops.OPS:
        if op.name == name:
            return op
    spec = Spec(
        body=Src0 * Src1 * C1,
        accum=minn,
        accum_init=C0,
        reference=lambda in0, in1, s0, s1, imm2: (
            np.asarray(in0, np.float32) * in1 * s1
        ),
    )
    row = dve_ops._CUSTOM_DVE_ROW_BASE + len(dve_ops.OPS)
    assert row < 0x20
    shas = {
        ver: DveOpSpec(name=name, opcode=row, uops=lower(spec, ver=ver),
                       rd1_en=_has_src1(spec)).sha(ver)
        for ver in ("v3", "v4")
    }
    op = dve_ops.DveOp(name, spec, subdim=False, uops_sha=shas)
    dve_ops.OPS.append(op)
    dve_ops.CUSTOM_DVE_SPECS[name] = spec
    dve_ops._SUB_OPCODE_FOR_NAME[name] = row
    return op


def _build_nc(reps=1):
    import contextlib

    ttr_min = _make_ttr_min()
    nc = bacc.Bacc("TRN2", target_bir_lowering=False, debug=False)
    xs = nc.dram_tensor("xs", [B, CCH, W, H], F32R, kind="ExternalInput").ap()
    wp = nc.dram_tensor("wp", [CCH, 128, KT, 3, V], F32R, kind="ExternalInput").ap()
    bp = nc.dram_tensor("bp", [CCH, 128, HP, 3, V], F32R, kind="ExternalInput").ap()
    o = nc.dram_tensor("o", [B, CCH, H, V], BF16, kind="ExternalOutput").ap()

    MULT = mybir.AluOpType.mult

    with tile.TileContext(nc) as tc:
        with (
            tc.tile_pool(name="const", bufs=1) as cpool,
            tc.tile_pool(name="wts", bufs=3) as wpool,
            tc.tile_pool(name="xb", bufs=3) as xpool,
            tc.tile_pool(name="sb", bufs=4) as sb,
            tc.tile_pool(name="ob", bufs=3) as ob,
            tc.tile_pool(name="ps", bufs=2, space="PSUM") as ps,
        ):
            ident = cpool.tile([128, 128], F32)
            make_identity(nc, ident[:])
            ident_r = cpool.tile([128, 128], F32R)
            nc.vector.tensor_copy(ident_r[:], ident[:])

            rep_ctx = (tc.For_i(0, reps, staggered_reset=True)
                       if reps != 1 else contextlib.nullcontext())
            with rep_ctx:
              for cc in range(CCH):
                # biases first (preloads need only these), then weights, then
                # x in two chunks so the first pairs' data matmuls start early.
                b_all = wpool.tile([128, HP, 3, V], F32R, tag="b")
                nc.sync.dma_start(b_all[:], bp[cc])
                w_all = wpool.tile([128, KT, 3, V], F32R, tag="w")
                nc.sync.dma_start(w_all[:], wp[cc])
                xc = xpool.tile([128, B, KT, H], F32R, tag="x")
                XSPLIT = 2
                nc.sync.dma_start(
                    xc[:, 0:XSPLIT],
                    xs[0:XSPLIT, cc].rearrange("b (p k) h -> p b k h", k=KT))
                nc.sync.dma_start(
                    xc[:, XSPLIT:B],
                    xs[XSPLIT:B, cc].rearrange("b (p k) h -> p b k h", k=KT))
                oc = ob.tile([128, B, HP, V], BF16, tag="oc")

                for b in range(B):
                    qb = ps.tile([128, 512], F32, tag="q", name="q")
                    kb = ps.tile([128, 512], F32, tag="k", name="k")
                    vb = ps.tile([128, 512], F32, tag="v", name="v", bufs=3)
                    nc.tensor.matmul(kb[:], ident_r[:], b_all[:, :, 1],
                                     start=True, stop=False)
                    nc.tensor.matmul(qb[:], ident_r[:], b_all[:, :, 0],
                                     start=True, stop=False)
                    nc.tensor.matmul(vb[:], ident_r[:], b_all[:, :, 2],
                                     start=True, stop=False)
                    # K first (unblocks the ScalarE evacuation), then Q, then V.
                    for j, bank in ((1, kb), (0, qb), (2, vb)):
                        for m in range(HP):
                            for kt in range(KT):
                                last = m == HP - 1 and kt == KT - 1
                                lhs = xc[:, b, kt, m * 128:(m + 1) * 128]
                                nc.tensor.matmul(bank[:, m * V:(m + 1) * V],
                                                 lhs, w_all[:, kt, j],
                                                 start=False, stop=last,
                                                 skip_group_check=True)

                    k_sb = sb.tile([128, 512], F32, tag="ksb", bufs=6)
                    nc.scalar.copy(k_sb[:], kb[:])
                    s_sb = sb.tile([128, 512], F32, tag="s", bufs=6)
                    mneg = sb.tile([128, HP], F32, tag="mneg")
                    # per-half -rowmax: the two halves' maxes can differ by
                    # more than exp's fp32 range, so a shared shift underflows
                    # a whole half to zero (sum=0 -> 1/0).
                    for m in range(HP):
                        nc.vector._custom_dve(
                            ttr_min,
                            out=s_sb[:, m * V:(m + 1) * V],
                            in0=qb[:, m * V:(m + 1) * V],
                            in1=k_sb[:, m * V:(m + 1) * V],
                            s0=3.0e38, s1=-1.0,
                            accum_out=mneg[:, m:m + 1],
                        )
                    p_sb = sb.tile([128, HP, V], F32, tag="p", bufs=6)
                    sums = sb.tile([128, HP], F32, tag="sums")
                    for m in range(HP):
                        nc.scalar.activation(
                            p_sb[:, m], s_sb[:, m * V:(m + 1) * V],
                            mybir.ActivationFunctionType.Exp,
                            bias=mneg[:, m:m + 1], scale=-1.0,
                            accum_out=sums[:, m:m + 1],
                        )
                    r_sb = sb.tile([128, HP], F32, tag="r")
                    nc.vector.reciprocal(r_sb[:], sums[:])
                    for m in range(HP):
                        nc.vector.scalar_tensor_tensor(
                            oc[:, b, m], p_sb[:, m], r_sb[:, m:m + 1],
                            vb[:, m * V:(m + 1) * V],
                            op0=MULT, op1=MULT)

                    # SWDGE on the idle Pool engine: its sem wait must not
                    # block the SP queue, which feeds the next channels'
                    # input loads. Per-pair so the tail DMA is small.
                    nc.gpsimd.dma_start(
                        o[b, cc].rearrange("(p m) v -> p m v", m=HP),
                        oc[:, b])
    nc.compile()
    return nc


def _host_xT(xc):
    """[B, CC, H, W] -> xT [B, CC, W, H'] with H' enumerating h as f = m*128 + j
    <-> h = 2j + m (matches the kernel's interleaved row mapping)."""
    B_, C_, H_, W_ = xc.shape
    xt = xc.transpose(0, 1, 3, 2)
    xt = xt.reshape(B_, C_, W_, H_ // 2, 2).swapaxes(-1, -2)
    return np.ascontiguousarray(xt.reshape(B_, C_, W_, H_))


def _shard_inputs(x, query_w, key_w, var_w, query_b, key_b, var_b):
    x = np.asarray(x, np.float32)
    qw = np.asarray(query_w, np.float32)
    kw = np.asarray(key_w, np.float32)
    vw = np.asarray(var_w, np.float32)
    qb = np.asarray(query_b, np.float32)
    kb = np.asarray(key_b, np.float32)
    vb = np.asarray(var_b, np.float32)
    in_maps = []
    for c in range(NCORES):
        sl = slice(c * CCH, (c + 1) * CCH)
        # weights: [CCH, W, 3, V] -> [CCH, 128, KT, 3, V], w = 2p + kt
        w3 = np.stack([qw[sl], kw[sl], vw[sl]], axis=2)
        wpk = np.ascontiguousarray(w3.reshape(CCH, 128, KT, 3, V))
        # biases: [CCH, H, 3, V] -> [CCH, 128, HP, 3, V], h = 2p + m
        b3 = np.stack([qb[sl], kb[sl], vb[sl]], axis=2)
        bpk = np.ascontiguousarray(b3.reshape(CCH, 128, HP, 3, V))
        in_maps.append({"xs": _host_xT(x[:, sl]), "wp": wpk, "bp": bpk})
    return in_maps


def _gather_output(results):
    out = np.empty((B, C, H, V), np.float32)
    for c in range(NCORES):
        out[:, c * CCH:(c + 1) * CCH] = np.asarray(
            results[c]["o"], dtype=np.float32)
    return out


def kernel(x, query_w, key_w, var_w, query_b, key_b, var_b):
    from concourse.bass_utils import run_bass_kernel_spmd

    in_maps = _shard_inputs(x, query_w, key_w, var_w, query_b, key_b, var_b)
    nc = _build_nc()
    res = run_bass_kernel_spmd(nc, in_maps, list(range(NCORES)))
    return _gather_output(res.results)


# revision 11
# speedup vs baseline: 1.1146x; 1.1146x over previous
"""AxileAttention Trainium2 kernel, v3 (self-contained).

Problem: x[8,64,256,256] fp32; per-channel weights *_w[64,256,256], biases *_b[64,256,256]:
    q = einsum("bchw,cwv->bchv", x, query_w) + query_b   (same for k, v)
    out = softmax(q*k, axis=-1) * v

Strategy (8 NeuronCores, SPMD): shard channel axis C=64 -> 8 channels/core.
Per (batch, channel) pair:
  * 3 PSUM banks Q=[q_m0|q_m1], K=[k_m0|k_m1], V=[v_m0|v_m1] (h = 2j+m
    interleave; partition j holds rows 2j, 2j+1). Biases preloaded via
    f32r identity matmuls; 12 f32r data matmuls accumulate on top.
  * ScalarE evacuates K -> SBUF; one fused 512-wide DVE op computes
    s_neg = -(q*k) and the union -rowmax; 2x ScalarE Exp (shared union max
    is a valid softmax shift) accumulate per-half row sums; DVE reciprocal;
    2x DVE scalar_tensor_tensor out = (p * 1/sum) * v reading v straight
    from PSUM, writing bf16.
  * DMAs are all HWDGE (nc.sync), batched per channel: x (all 8 batches),
    packed weights [128,2,3,256], packed biases [128,2,3,256], bf16 output
    (all 8 batches). f32r is bit-identical to fp32 so inputs are declared
    f32r in DRAM and copied raw (no SWDGE cast DMAs).
"""
import sys

sys.path.insert(0, "/opt/trn_rl_repo")

import numpy as np

import concourse.bacc as bacc
import concourse.tile as tile
import concourse.dve_ops as dve_ops
from concourse import mybir
from concourse.masks import make_identity
from concourse.dve_spec import C0, C1, Spec, Src0, Src1, lower, minn, _has_src1
from concourse.dve_uop import DveOpSpec

F32 = mybir.dt.float32
F32R = mybir.dt.float32r
BF16 = mybir.dt.bfloat16

B = 8        # batch
C = 64       # channels total
CCH = 8      # channels per core
NCORES = 8
HP = 2       # h partition-tiles (h = 2j + m interleave)
KT = 2       # w partition-tiles (w = 2p + k interleave)
H = W = V = 256


def _make_ttr_min():
    """Custom DVE op: out = (in0*in1)*s1 ; accum_out = min(s0, row-min of out).
    Called with s1=-1, s0=+BIG: out = -(q*k), accum = -rowmax(q*k)."""
    name = "TTR_MIN_NEG_ANT"
    for op in dve_ops.OPS:
        if op.name == name:
            return op
    spec = Spec(
        body=Src0 * Src1 * C1,
        accum=minn,
        accum_init=C0,
        reference=lambda in0, in1, s0, s1, imm2: (
            np.asarray(in0, np.float32) * in1 * s1
        ),
    )
    row = dve_ops._CUSTOM_DVE_ROW_BASE + len(dve_ops.OPS)
    assert row < 0x20
    shas = {
        ver: DveOpSpec(name=name, opcode=row, uops=lower(spec, ver=ver),
                       rd1_en=_has_src1(spec)).sha(ver)
        for ver in ("v3", "v4")
    }
    op = dve_ops.DveOp(name, spec, subdim=False, uops_sha=shas)
    dve_ops.OPS.append(op)
    dve_ops.CUSTOM_DVE_SPECS[name] = spec
    dve_ops._SUB_OPCODE_FOR_NAME[name] = row
    return op


def _build_nc(reps=1):
    import contextlib

    ttr_min = _make_ttr_min()
    nc = bacc.Bacc("TRN2", target_bir_lowering=False, debug=False)
    xs = nc.dram_tensor("xs", [B, CCH, W, H], F32R, kind="ExternalInput").ap()
    wp = nc.dram_tensor("wp", [CCH, 128, KT, 3, V], F32R, kind="ExternalInput").ap()
    bp = nc.dram_tensor("bp", [CCH, 128, HP, 3, V], F32R, kind="ExternalInput").ap()
    o = nc.dram_tensor("o", [B, CCH, H, V], BF16, kind="ExternalOutput").ap()

    MULT = mybir.AluOpType.mult

    with tile.TileContext(nc) as tc:
        with (
            tc.tile_pool(name="const", bufs=1) as cpool,
            tc.tile_pool(name="wts", bufs=2) as wpool,
            tc.tile_pool(name="xb", bufs=2) as xpool,
            tc.tile_pool(name="sb", bufs=4) as sb,
            tc.tile_pool(name="ob", bufs=2) as ob,
            tc.tile_pool(name="ps", bufs=2, space="PSUM") as ps,
        ):
            ident = cpool.tile([128, 128], F32)
            make_identity(nc, ident[:])
            ident_r = cpool.tile([128, 128], F32R)
            nc.vector.tensor_copy(ident_r[:], ident[:])

            rep_ctx = tc.For_i(0, reps) if reps != 1 else contextlib.nullcontext()
            with rep_ctx:
              for cc in range(CCH):
                # biases first (preloads need only these), then weights, then
                # x in two chunks so the first pairs' data matmuls start early.
                b_all = wpool.tile([128, HP, 3, V], F32R, tag="b")
                nc.sync.dma_start(b_all[:], bp[cc])
                w_all = wpool.tile([128, KT, 3, V], F32R, tag="w")
                nc.sync.dma_start(w_all[:], wp[cc])
                xc = xpool.tile([128, B, KT, H], F32R, tag="x")
                XSPLIT = 2
                nc.sync.dma_start(
                    xc[:, 0:XSPLIT],
                    xs[0:XSPLIT, cc].rearrange("b (p k) h -> p b k h", k=KT))
                nc.sync.dma_start(
                    xc[:, XSPLIT:B],
                    xs[XSPLIT:B, cc].rearrange("b (p k) h -> p b k h", k=KT))
                oc = ob.tile([128, B, HP, V], BF16, tag="oc")

                for b in range(B):
                    qb = ps.tile([128, 512], F32, tag="q", name="q")
                    kb = ps.tile([128, 512], F32, tag="k", name="k")
                    vb = ps.tile([128, 512], F32, tag="v", name="v", bufs=4)
                    nc.tensor.matmul(kb[:], ident_r[:], b_all[:, :, 1],
                                     start=True, stop=False)
                    nc.tensor.matmul(qb[:], ident_r[:], b_all[:, :, 0],
                                     start=True, stop=False)
                    nc.tensor.matmul(vb[:], ident_r[:], b_all[:, :, 2],
                                     start=True, stop=False)
                    # K first (unblocks the ScalarE evacuation), then Q, then V.
                    for j, bank in ((1, kb), (0, qb), (2, vb)):
                        for m in range(HP):
                            for kt in range(KT):
                                last = m == HP - 1 and kt == KT - 1
                                lhs = xc[:, b, kt, m * 128:(m + 1) * 128]
                                nc.tensor.matmul(bank[:, m * V:(m + 1) * V],
                                                 lhs, w_all[:, kt, j],
                                                 start=False, stop=last,
                                                 skip_group_check=True)

                    k_sb = sb.tile([128, 512], F32, tag="ksb", bufs=6)
                    nc.scalar.copy(k_sb[:], kb[:])
                    s_sb = sb.tile([128, 512], F32, tag="s", bufs=6)
                    mneg = sb.tile([128, HP], F32, tag="mneg")
                    # per-half -rowmax: the two halves' maxes can differ by
                    # more than exp's fp32 range, so a shared shift underflows
                    # a whole half to zero (sum=0 -> 1/0).
                    for m in range(HP):
                        nc.vector._custom_dve(
                            ttr_min,
                            out=s_sb[:, m * V:(m + 1) * V],
                            in0=qb[:, m * V:(m + 1) * V],
                            in1=k_sb[:, m * V:(m + 1) * V],
                            s0=3.0e38, s1=-1.0,
                            accum_out=mneg[:, m:m + 1],
                        )
                    p_sb = sb.tile([128, HP, V], F32, tag="p", bufs=6)
                    sums = sb.tile([128, HP], F32, tag="sums")
                    for m in range(HP):
                        nc.scalar.activation(
                            p_sb[:, m], s_sb[:, m * V:(m + 1) * V],
                            mybir.ActivationFunctionType.Exp,
                            bias=mneg[:, m:m + 1], scale=-1.0,
                            accum_out=sums[:, m:m + 1],
                        )
                    r_sb = sb.tile([128, HP], F32, tag="r")
                    nc.vector.reciprocal(r_sb[:], sums[:])
                    for m in range(HP):
                        nc.vector.scalar_tensor_tensor(
                            oc[:, b, m], p_sb[:, m], r_sb[:, m:m + 1],
                            vb[:, m * V:(m + 1) * V],
                            op0=MULT, op1=MULT)

                    # SWDGE on the idle Pool engine: its sem wait must not
                    # block the SP queue, which feeds the next channels'
                    # input loads. Per-pair so the tail DMA is small.
                    nc.gpsimd.dma_start(
                        o[b, cc].rearrange("(p m) v -> p m v", m=HP),
                        oc[:, b])
    nc.compile()
    return nc


def _host_xT(xc):
    """[B, CC, H, W] -> xT [B, CC, W, H'] with H' enumerating h as f = m*128 + j
    <-> h = 2j + m (matches the kernel's interleaved row mapping)."""
    B_, C_, H_, W_ = xc.shape
    xt = xc.transpose(0, 1, 3, 2)
    xt = xt.reshape(B_, C_, W_, H_ // 2, 2).swapaxes(-1, -2)
    return np.ascontiguousarray(xt.reshape(B_, C_, W_, H_))


def _shard_inputs(x, query_w, key_w, var_w, query_b, key_b, var_b):
    x = np.asarray(x, np.float32)
    qw = np.asarray(query_w, np.float32)
    kw = np.asarray(key_w, np.float32)
    vw = np.asarray(var_w, np.float32)
    qb = np.asarray(query_b, np.float32)
    kb = np.asarray(key_b, np.float32)
    vb = np.asarray(var_b, np.float32)
    in_maps = []
    for c in range(NCORES):
        sl = slice(c * CCH, (c + 1) * CCH)
        # weights: [CCH, W, 3, V] -> [CCH, 128, KT, 3, V], w = 2p + kt
        w3 = np.stack([qw[sl], kw[sl], vw[sl]], axis=2)
        wpk = np.ascontiguousarray(w3.reshape(CCH, 128, KT, 3, V))
        # biases: [CCH, H, 3, V] -> [CCH, 128, HP, 3, V], h = 2p + m
        b3 = np.stack([qb[sl], kb[sl], vb[sl]], axis=2)
        bpk = np.ascontiguousarray(b3.reshape(CCH, 128, HP, 3, V))
        in_maps.append({"xs": _host_xT(x[:, sl]), "wp": wpk, "bp": bpk})
    return in_maps


def _gather_output(results):
    out = np.empty((B, C, H, V), np.float32)
    for c in range(NCORES):
        out[:, c * CCH:(c + 1) * CCH] = np.asarray(
            results[c]["o"], dtype=np.float32)
    return out


def kernel(x, query_w, key_w, var_w, query_b, key_b, var_b):
    from concourse.bass_utils import run_bass_kernel_spmd

    in_maps = _shard_inputs(x, query_w, key_w, var_w, query_b, key_b, var_b)
    nc = _build_nc()
    res = run_bass_kernel_spmd(nc, in_maps, list(range(NCORES)))
    return _gather_output(res.results)
